# revision 3
# baseline (speedup 1.0000x reference)
"""Trainium2 Bass kernel for nn_DecoderMultiHeadedAttention_58102317580621.

Multi-head decoder (causal) attention, B=2, T=2048, C=128 (per-head dim),
H=16 heads, fused qkv projections + causal softmax attention + output fc.

Sharding: core = b*4 + g owns batch b and heads [4g, 4g+4).  Each core
computes, for its 4 heads: qT/kT/v projections, causal-masked softmax
attention (transposed layout, denominator via ones-matmul), and the partial
fc contraction summed over its heads on-chip (PSUM).  Host sums the 4
per-core partials per batch and adds the bias.

Layouts (per core, all on-chip tensors have 128 partitions):
  xT      (C, T)   = x[b].T                    fp32r
  qT_h    (C, T)   = (x @ Wq_h).T  = Wq_h.T @ x.T   (matmul lhsT=Wq_h, rhs=xT)
  kT_h    (C, T)   analogous
  v       (T, C)   natural, bf16, 4 heads packed:  v[:, si, j, :]
  scoresT (s, t)   = kT_h[:, s].T @ qT_h[:, t]  (contraction over per-head c')
  exp     bf16, causal mask applied multiplicatively post-exp (no max
           subtraction: |logit| <= ~6 so exp is safe in fp32/bf16)
  out'    (C, t)   = sum_s v[s,c] * exp[s,t]   (PSUM accum over s blocks)
  den     (*, t)   = sum_s exp[s,t] broadcast over partitions via ones-matmul
  y'      (OUT, t) = sum_heads Wfc_h.T @ (out' * 1/den)   (PSUM accum)
"""

import numpy as np
import ml_dtypes

B, T, C, H, OUT = 2, 2048, 128, 16, 128
NCORES = 8
HPC = 4          # heads per core
TCW = 512        # t-chunk width
NTC = T // TCW   # 4
SB = 128         # s-block (PE contraction tile)
GW = 2           # s-blocks per score super-tile (128 x GW*TCW psum)
INV_SQRT_C = float(1.0 / np.sqrt(C))

_CACHE = {}


def _make_masks():
    # masks[r][p, t] = 1 if (128*r + p) <= t else 0   (keep s <= t)
    p = np.arange(SB)[:, None]
    t = np.arange(TCW)[None, :]
    m = np.stack([(p + SB * r <= t) for r in range(4)]).astype(ml_dtypes.bfloat16)
    return np.ascontiguousarray(m)


def _build():
    import concourse.tile as tile
    from concourse import bacc, mybir

    f32 = mybir.dt.float32
    f32r = mybir.dt.float32r
    bf16 = mybir.dt.bfloat16
    EXPF = mybir.ActivationFunctionType.Exp

    nc = bacc.Bacc("TRN2", target_bir_lowering=False, debug=False,
                   num_devices=NCORES)
    xT = nc.dram_tensor("xT", [C, T], f32r, kind="ExternalInput").ap()
    wq = nc.dram_tensor("wq", [HPC, C, C], f32r, kind="ExternalInput").ap()
    wk = nc.dram_tensor("wk", [HPC, C, C], f32r, kind="ExternalInput").ap()
    wv = nc.dram_tensor("wv", [C, HPC * C], f32r, kind="ExternalInput").ap()
    wfc = nc.dram_tensor("wfc", [HPC, C, OUT], f32r, kind="ExternalInput").ap()
    masks = nc.dram_tensor("masks", [4, SB, TCW], bf16, kind="ExternalInput").ap()
    y = nc.dram_tensor("y", [OUT, T], f32, kind="ExternalOutput").ap()

    with tile.TileContext(nc) as tc:
        with (
            tc.tile_pool(name="const", bufs=1) as const,
            tc.tile_pool(name="big", bufs=1) as big,
            tc.tile_pool(name="super", bufs=2, space="PSUM") as psup,
            tc.tile_pool(name="psacc", bufs=1, space="PSUM") as psacc,
            tc.tile_pool(name="psy", bufs=2, space="PSUM") as psy,
            tc.tile_pool(name="exp", bufs=10) as expp,
            tc.tile_pool(name="small", bufs=3) as small,
        ):
            # ---- constants / inputs to SBUF ----
            xT_sb = big.tile([C, T], f32r, tag="xT")
            nc.sync.dma_start(xT_sb[:], xT[:])
            wq_sb = const.tile([C, HPC, C], f32r, tag="wq")
            wk_sb = const.tile([C, HPC, C], f32r, tag="wk")
            wfc_sb = const.tile([C, HPC, OUT], f32r, tag="wfc")
            for h in range(HPC):
                nc.sync.dma_start(wq_sb[:, h, :], wq[h])
                nc.sync.dma_start(wk_sb[:, h, :], wk[h])
                nc.sync.dma_start(wfc_sb[:, h, :], wfc[h])
            wv_sb = const.tile([C, HPC * C], f32r, tag="wv")
            nc.sync.dma_start(wv_sb[:], wv[:])
            mask_sb = const.tile([SB, 4 * TCW], bf16, tag="mask")
            for r in range(4):
                nc.sync.dma_start(mask_sb[:, r * TCW:(r + 1) * TCW], masks[r])
            ones_sb = const.tile([SB, SB], bf16, tag="ones")
            nc.vector.memset(ones_sb[:], 1.0)

            # ---- projections ----
            # v: natural layout (t-part, [heads, c]-free), bf16
            v_sb = big.tile([SB, T // SB, HPC, SB], bf16, tag="v")
            for sv in range(T // SB // GW):
                ps = psup.tile([C, GW * HPC * SB], f32, tag="super")
                for u in range(GW):
                    tb = GW * sv + u
                    nc.tensor.matmul(
                        ps[:, u * 512:(u + 1) * 512],
                        lhsT=xT_sb[:, tb * SB:(tb + 1) * SB],
                        rhs=wv_sb[:],
                        start=True, stop=True,
                    )
                nc.vector.tensor_copy(v_sb[:, GW * sv:GW * (sv + 1), :, :], ps[:])

            # qT / kT per head (c'-part, t-free), fp32r
            qT_sb = [big.tile([C, T], f32r, tag=f"qT{h}", name=f"qT{h}")
                     for h in range(HPC)]
            kT_sb = [big.tile([C, T], f32r, tag=f"kT{h}", name=f"kT{h}")
                     for h in range(HPC)]
            for h in range(HPC):
                for w_sb, dst in ((wq_sb, qT_sb[h]), (wk_sb, kT_sb[h])):
                    for sv in range(2):
                        ps = psup.tile([C, 1024], f32, tag="super")
                        for u in range(2):
                            tci = 2 * sv + u
                            nc.tensor.matmul(
                                ps[:, u * TCW:(u + 1) * TCW],
                                lhsT=w_sb[:, h, :],
                                rhs=xT_sb[:, tci * TCW:(tci + 1) * TCW],
                                start=True, stop=True,
                            )
                        nc.vector.tensor_copy(dst[:, sv * 1024:(sv + 1) * 1024], ps[:])

            # ---- attention + fc ----
            for tci in range(NTC):
                yps = psy.tile([OUT, TCW], f32, tag="yps")
                for j in range(HPC):
                    n_si = 4 * tci + 4
                    n_g = n_si // GW
                    # scores + exp (+ mask on the two diagonal groups)
                    exps = []
                    for G in range(n_g):
                        ps = psup.tile([C, GW * TCW], f32, tag="super")
                        for u in range(GW):
                            si = GW * G + u
                            nc.tensor.matmul(
                                ps[:, u * TCW:(u + 1) * TCW],
                                lhsT=kT_sb[j][:, si * SB:(si + 1) * SB],
                                rhs=qT_sb[j][:, tci * TCW:(tci + 1) * TCW],
                                start=True, stop=True,
                            )
                        ex = expp.tile([C, GW * TCW], bf16, tag="exp")
                        nc.scalar.activation(out=ex[:], in_=ps[:], func=EXPF,
                                             scale=INV_SQRT_C)
                        if G >= n_g - 2:
                            r0 = (G - (n_g - 2)) * GW
                            nc.vector.tensor_mul(
                                ex[:], ex[:],
                                mask_sb[:, r0 * TCW:(r0 + GW) * TCW],
                            )
                        exps.append(ex)
                    # out' accumulation over s blocks
                    outp = psacc.tile([C, TCW], f32, tag="outp")
                    k = 0
                    for G in range(n_g):
                        for u in range(GW):
                            si = GW * G + u
                            nc.tensor.matmul(
                                outp[:],
                                lhsT=v_sb[:, si, j, :],
                                rhs=exps[G][:, u * TCW:(u + 1) * TCW],
                                start=(k == 0), stop=(k == n_si - 1),
                            )
                            k += 1
                    # denominator (broadcast over partitions via ones)
                    denp = psacc.tile([C, TCW], f32, tag="denp")
                    k = 0
                    for G in range(n_g):
                        for u in range(GW):
                            nc.tensor.matmul(
                                denp[:],
                                lhsT=ones_sb[:],
                                rhs=exps[G][:, u * TCW:(u + 1) * TCW],
                                start=(k == 0), stop=(k == n_si - 1),
                            )
                            k += 1
                    # normalize:  outn = out' * (1/den)   (bf16)
                    recip = small.tile([C, TCW], f32, tag="recip")
                    nc.vector.reciprocal(recip[:], denp[:])
                    outn = small.tile([C, TCW], f32r, tag="outn")
                    nc.vector.tensor_mul(outn[:], outp[:], recip[:])
                    # fc partial, accumulated across this core's heads
                    nc.tensor.matmul(
                        yps[:],
                        lhsT=wfc_sb[:, j, :],
                        rhs=outn[:],
                        start=(j == 0), stop=(j == HPC - 1),
                    )
                ysb = small.tile([OUT, TCW], f32, tag="ysb")
                nc.vector.tensor_copy(ysb[:], yps[:])
                nc.sync.dma_start(y[:, tci * TCW:(tci + 1) * TCW], ysb[:])

    nc.compile()
    return nc


def _in_maps(x, Wq, Wk, Wv, Wfc):
    masks = _make_masks()
    maps = []
    for core in range(NCORES):
        b, g = divmod(core, NCORES // B)
        heads = [HPC * g + j for j in range(HPC)]
        maps.append({
            "xT": np.ascontiguousarray(x[b].T),
            "wq": np.ascontiguousarray(np.stack([Wq[:, h::H] for h in heads])),
            "wk": np.ascontiguousarray(np.stack([Wk[:, h::H] for h in heads])),
            "wv": np.ascontiguousarray(
                np.concatenate([Wv[:, h::H] for h in heads], axis=1)),
            "wfc": np.ascontiguousarray(
                np.stack([Wfc[h * C:(h + 1) * C, :] for h in heads])),
            "masks": masks,
        })
    return maps


def kernel(**inputs):
    from concourse.bass_utils import run_bass_kernel_spmd

    x = np.asarray(inputs["x"], dtype=np.float32)
    Wq = np.asarray(inputs["Wq"], dtype=np.float32)
    Wk = np.asarray(inputs["Wk"], dtype=np.float32)
    Wv = np.asarray(inputs["Wv"], dtype=np.float32)
    Wfc = np.asarray(inputs["Wfc"], dtype=np.float32)
    bfc = np.asarray(inputs["bfc"], dtype=np.float32)

    if "nc" not in _CACHE:
        _CACHE["nc"] = _build()
    nc = _CACHE["nc"]

    res = run_bass_kernel_spmd(nc, _in_maps(x, Wq, Wk, Wv, Wfc),
                               core_ids=list(range(NCORES)))
    out = np.empty((B, T, OUT), dtype=np.float32)
    for b in range(B):
        acc = bfc[:, None].copy()
        for g in range(NCORES // B):
            acc = acc + res.results[(NCORES // B) * b + g]["y"]
        out[b] = acc.T
    return out


# revision 5
# speedup vs baseline: 2695.8877x; 2695.8877x over previous
"""Trainium2 Bass kernel for nn_DecoderMultiHeadedAttention_58102317580621.

Multi-head decoder (causal) attention, B=2, T=2048, C=128 (per-head dim),
H=16 heads, fused qkv projections + causal softmax attention + output fc.

Sharding: core = b*4 + g owns batch b and heads [4g, 4g+4).  Each core
computes, for its 4 heads: qT/kT/v projections, causal-masked softmax
attention (transposed layout, denominator via ones-matmul), and the partial
fc contraction summed over its heads on-chip (PSUM).  Host sums the 4
per-core partials per batch and adds the bias.

Layouts (per core, all on-chip tensors have 128 partitions):
  xT      (C, T)   = x[b].T                    fp32r
  qT_h    (C, T)   = (x @ Wq_h).T  = Wq_h.T @ x.T   (matmul lhsT=Wq_h, rhs=xT)
  kT_h    (C, T)   analogous
  v       (T, C)   natural, bf16, 4 heads packed:  v[:, si, j, :]
  scoresT (s, t)   = kT_h[:, s].T @ qT_h[:, t]  (contraction over per-head c')
  exp     bf16, causal mask applied multiplicatively post-exp (no max
           subtraction: |logit| <= ~6 so exp is safe in fp32/bf16)
  out'    (C, t)   = sum_s v[s,c] * exp[s,t]   (PSUM accum over s blocks)
  den     (*, t)   = sum_s exp[s,t] broadcast over partitions via ones-matmul
  y'      (OUT, t) = sum_heads Wfc_h.T @ (out' * 1/den)   (PSUM accum)
"""

import numpy as np
import ml_dtypes

B, T, C, H, OUT = 2, 2048, 128, 16, 128
NCORES = 8
HPC = 4          # heads per core
TCW = 512        # t-chunk width
NTC = T // TCW   # 4
SB = 128         # s-block (PE contraction tile)
GW = 2           # s-blocks per score super-tile (128 x GW*TCW psum)
INV_SQRT_C = float(1.0 / np.sqrt(C))

_CACHE = {}


def _make_masks():
    # masks[r][p, t] = 1 if (128*r + p) <= t else 0   (keep s <= t)
    p = np.arange(SB)[:, None]
    t = np.arange(TCW)[None, :]
    m = np.stack([(p + SB * r <= t) for r in range(4)]).astype(ml_dtypes.bfloat16)
    return np.ascontiguousarray(m)


def _repeat_loop(tc, mybir, repeat):
    # Timing builds wrap the whole body in a hardware loop so per-iteration
    # device time can be measured from wall-clock deltas.
    import contextlib
    if repeat <= 1:
        return contextlib.nullcontext()
    return tc.For_i(0, repeat, 1,
                    hint_engines=(mybir.EngineType.PE,
                                  mybir.EngineType.Activation,
                                  mybir.EngineType.DVE))


def _build(repeat=1):
    import concourse.tile as tile
    from concourse import bacc, mybir

    f32 = mybir.dt.float32
    f32r = mybir.dt.float32r
    bf16 = mybir.dt.bfloat16
    EXPF = mybir.ActivationFunctionType.Exp

    nc = bacc.Bacc("TRN2", target_bir_lowering=False, debug=False,
                   num_devices=NCORES)
    xT = nc.dram_tensor("xT", [C, T], f32r, kind="ExternalInput").ap()
    wq = nc.dram_tensor("wq", [HPC, C, C], f32r, kind="ExternalInput").ap()
    wk = nc.dram_tensor("wk", [HPC, C, C], f32r, kind="ExternalInput").ap()
    wv = nc.dram_tensor("wv", [C, HPC * C], f32r, kind="ExternalInput").ap()
    wfc = nc.dram_tensor("wfc", [HPC, C, OUT], f32r, kind="ExternalInput").ap()
    masks = nc.dram_tensor("masks", [4, SB, TCW], bf16, kind="ExternalInput").ap()
    y = nc.dram_tensor("y", [OUT, T], f32, kind="ExternalOutput").ap()

    with tile.TileContext(nc) as tc:
        with (
            tc.tile_pool(name="const", bufs=1) as const,
            tc.tile_pool(name="big", bufs=1) as big,
            tc.tile_pool(name="super", bufs=2, space="PSUM") as psup,
            tc.tile_pool(name="psacc", bufs=1, space="PSUM") as psacc,
            tc.tile_pool(name="psy", bufs=2, space="PSUM") as psy,
            tc.tile_pool(name="exp", bufs=10) as expp,
            tc.tile_pool(name="small", bufs=3) as small,
            _repeat_loop(tc, mybir, repeat),
        ):
            # ---- constants / inputs to SBUF ----
            xT_sb = big.tile([C, T], f32r, tag="xT")
            nc.sync.dma_start(xT_sb[:], xT[:])
            wq_sb = const.tile([C, HPC, C], f32r, tag="wq")
            wk_sb = const.tile([C, HPC, C], f32r, tag="wk")
            wfc_sb = const.tile([C, HPC, OUT], f32r, tag="wfc")
            for h in range(HPC):
                nc.sync.dma_start(wq_sb[:, h, :], wq[h])
                nc.sync.dma_start(wk_sb[:, h, :], wk[h])
                nc.sync.dma_start(wfc_sb[:, h, :], wfc[h])
            wv_sb = const.tile([C, HPC * C], f32r, tag="wv")
            nc.sync.dma_start(wv_sb[:], wv[:])
            mask_sb = const.tile([SB, 4 * TCW], bf16, tag="mask")
            for r in range(4):
                nc.sync.dma_start(mask_sb[:, r * TCW:(r + 1) * TCW], masks[r])
            ones_sb = const.tile([SB, SB], bf16, tag="ones")
            nc.vector.memset(ones_sb[:], 1.0)

            # ---- projections ----
            # v: natural layout (t-part, [heads, c]-free), bf16
            v_sb = big.tile([SB, T // SB, HPC, SB], bf16, tag="v")
            for sv in range(T // SB // GW):
                ps = psup.tile([C, GW * HPC * SB], f32, tag="super")
                for u in range(GW):
                    tb = GW * sv + u
                    nc.tensor.matmul(
                        ps[:, u * 512:(u + 1) * 512],
                        lhsT=xT_sb[:, tb * SB:(tb + 1) * SB],
                        rhs=wv_sb[:],
                        start=True, stop=True,
                    )
                nc.vector.tensor_copy(v_sb[:, GW * sv:GW * (sv + 1), :, :], ps[:])

            # qT / kT per head (c'-part, t-free), fp32r
            qT_sb = [big.tile([C, T], f32r, tag=f"qT{h}", name=f"qT{h}")
                     for h in range(HPC)]
            kT_sb = [big.tile([C, T], f32r, tag=f"kT{h}", name=f"kT{h}")
                     for h in range(HPC)]
            for h in range(HPC):
                for w_sb, dst in ((wq_sb, qT_sb[h]), (wk_sb, kT_sb[h])):
                    for sv in range(2):
                        ps = psup.tile([C, 1024], f32, tag="super")
                        for u in range(2):
                            tci = 2 * sv + u
                            nc.tensor.matmul(
                                ps[:, u * TCW:(u + 1) * TCW],
                                lhsT=w_sb[:, h, :],
                                rhs=xT_sb[:, tci * TCW:(tci + 1) * TCW],
                                start=True, stop=True,
                            )
                        nc.vector.tensor_copy(dst[:, sv * 1024:(sv + 1) * 1024], ps[:])

            # ---- attention + fc ----
            for tci in range(NTC):
                yps = psy.tile([OUT, TCW], f32, tag="yps")
                for j in range(HPC):
                    n_si = 4 * tci + 4
                    n_g = n_si // GW
                    # scores + exp (+ mask on the two diagonal groups)
                    exps = []
                    for G in range(n_g):
                        ps = psup.tile([C, GW * TCW], f32, tag="super")
                        for u in range(GW):
                            si = GW * G + u
                            nc.tensor.matmul(
                                ps[:, u * TCW:(u + 1) * TCW],
                                lhsT=kT_sb[j][:, si * SB:(si + 1) * SB],
                                rhs=qT_sb[j][:, tci * TCW:(tci + 1) * TCW],
                                start=True, stop=True,
                            )
                        ex = expp.tile([C, GW * TCW], bf16, tag="exp")
                        nc.scalar.activation(out=ex[:], in_=ps[:], func=EXPF,
                                             scale=INV_SQRT_C)
                        if G >= n_g - 2:
                            r0 = (G - (n_g - 2)) * GW
                            nc.vector.tensor_mul(
                                ex[:], ex[:],
                                mask_sb[:, r0 * TCW:(r0 + GW) * TCW],
                            )
                        exps.append(ex)
                    # out' accumulation over s blocks
                    outp = psacc.tile([C, TCW], f32, tag="outp")
                    k = 0
                    for G in range(n_g):
                        for u in range(GW):
                            si = GW * G + u
                            nc.tensor.matmul(
                                outp[:],
                                lhsT=v_sb[:, si, j, :],
                                rhs=exps[G][:, u * TCW:(u + 1) * TCW],
                                start=(k == 0), stop=(k == n_si - 1),
                            )
                            k += 1
                    # denominator (broadcast over partitions via ones)
                    denp = psacc.tile([C, TCW], f32, tag="denp")
                    k = 0
                    for G in range(n_g):
                        for u in range(GW):
                            nc.tensor.matmul(
                                denp[:],
                                lhsT=ones_sb[:],
                                rhs=exps[G][:, u * TCW:(u + 1) * TCW],
                                start=(k == 0), stop=(k == n_si - 1),
                            )
                            k += 1
                    # normalize:  outn = out' * (1/den)   (bf16)
                    recip = small.tile([C, TCW], f32, tag="recip")
                    nc.vector.reciprocal(recip[:], denp[:])
                    outn = small.tile([C, TCW], f32r, tag="outn")
                    nc.vector.tensor_mul(outn[:], outp[:], recip[:])
                    # fc partial, accumulated across this core's heads
                    nc.tensor.matmul(
                        yps[:],
                        lhsT=wfc_sb[:, j, :],
                        rhs=outn[:],
                        start=(j == 0), stop=(j == HPC - 1),
                    )
                ysb = small.tile([OUT, TCW], f32, tag="ysb")
                nc.vector.tensor_copy(ysb[:], yps[:])
                nc.sync.dma_start(y[:, tci * TCW:(tci + 1) * TCW], ysb[:])

    nc.compile()
    return nc


def _in_maps(x, Wq, Wk, Wv, Wfc):
    masks = _make_masks()
    maps = []
    for core in range(NCORES):
        b, g = divmod(core, NCORES // B)
        heads = [HPC * g + j for j in range(HPC)]
        maps.append({
            "xT": np.ascontiguousarray(x[b].T),
            "wq": np.ascontiguousarray(np.stack([Wq[:, h::H] for h in heads])),
            "wk": np.ascontiguousarray(np.stack([Wk[:, h::H] for h in heads])),
            "wv": np.ascontiguousarray(
                np.concatenate([Wv[:, h::H] for h in heads], axis=1)),
            "wfc": np.ascontiguousarray(
                np.stack([Wfc[h * C:(h + 1) * C, :] for h in heads])),
            "masks": masks,
        })
    return maps


def kernel(**inputs):
    from concourse.bass_utils import run_bass_kernel_spmd

    x = np.asarray(inputs["x"], dtype=np.float32)
    Wq = np.asarray(inputs["Wq"], dtype=np.float32)
    Wk = np.asarray(inputs["Wk"], dtype=np.float32)
    Wv = np.asarray(inputs["Wv"], dtype=np.float32)
    Wfc = np.asarray(inputs["Wfc"], dtype=np.float32)
    bfc = np.asarray(inputs["bfc"], dtype=np.float32)

    if "nc" not in _CACHE:
        _CACHE["nc"] = _build()
    nc = _CACHE["nc"]

    res = run_bass_kernel_spmd(nc, _in_maps(x, Wq, Wk, Wv, Wfc),
                               core_ids=list(range(NCORES)))
    out = np.empty((B, T, OUT), dtype=np.float32)
    for b in range(B):
        acc = bfc[:, None].copy()
        for g in range(NCORES // B):
            acc = acc + res.results[(NCORES // B) * b + g]["y"]
        out[b] = acc.T
    return out


# revision 11
# speedup vs baseline: 3024.1383x; 1.1218x over previous
"""Trainium2 Bass kernel for nn_DecoderMultiHeadedAttention_58102317580621.

Multi-head decoder (causal) attention, B=2, T=2048, C=128 (per-head dim),
H=16 heads, fused qkv projections + causal softmax attention + output fc.

Sharding: core = b*4 + g owns batch b and heads [4g, 4g+4).  Each core
computes, for its 4 heads: qT/kT/v projections, causal-masked softmax
attention (transposed layout, denominator via ones-matmul), and the partial
fc contraction summed over its heads on-chip (PSUM).  Host sums the 4
per-core partials per batch and adds the bias.

Layouts (per core, all on-chip tensors have 128 partitions):
  xT      (C, T)   = x[b].T                    fp32r
  qT_h    (C, T)   = (x @ Wq_h).T  = Wq_h.T @ x.T   (matmul lhsT=Wq_h, rhs=xT)
  kT_h    (C, T)   analogous
  v       (T, C)   natural, bf16, 4 heads packed:  v[:, si, j, :]
  scoresT (s, t)   = kT_h[:, s].T @ qT_h[:, t]  (contraction over per-head c')
  exp     bf16, causal mask applied multiplicatively post-exp (no max
           subtraction: |logit| <= ~6 so exp is safe in fp32/bf16)
  out'    (C, t)   = sum_s v[s,c] * exp[s,t]   (PSUM accum over s blocks)
  den     (*, t)   = sum_s exp[s,t] broadcast over partitions via ones-matmul
  y'      (OUT, t) = sum_heads Wfc_h.T @ (out' * 1/den)   (PSUM accum)
"""

import numpy as np
import ml_dtypes

B, T, C, H, OUT = 2, 2048, 128, 16, 128
NCORES = 8
HPC = 4          # heads per core
TCW = 512        # t-chunk width
NTC = T // TCW   # 4
SB = 128         # s-block (PE contraction tile)
GW = 2           # s-blocks per score super-tile (128 x GW*TCW psum)
INV_SQRT_C = float(1.0 / np.sqrt(C))

_CACHE = {}


def _make_masks():
    # tri[p, t'] = 1 if p <= t' else 0  (triangle for the diagonal 128-block)
    p = np.arange(SB)[:, None]
    t = np.arange(SB)[None, :]
    return np.ascontiguousarray((p <= t).astype(ml_dtypes.bfloat16))


def _repeat_loop(tc, mybir, repeat):
    # Timing builds wrap the whole body in a hardware loop so per-iteration
    # device time can be measured from wall-clock deltas.
    import contextlib
    if repeat <= 1:
        return contextlib.nullcontext()
    return tc.For_i(0, repeat, 1,
                    hint_engines=(mybir.EngineType.PE,
                                  mybir.EngineType.Activation,
                                  mybir.EngineType.DVE))


def _build(repeat=1, with_den=True, with_av=True, with_scores=True):
    import concourse.tile as tile
    from concourse import bacc, mybir

    f32 = mybir.dt.float32
    f32r = mybir.dt.float32r
    bf16 = mybir.dt.bfloat16
    EXPF = mybir.ActivationFunctionType.Exp

    nc = bacc.Bacc("TRN2", target_bir_lowering=False, debug=False,
                   num_devices=NCORES)
    xT = nc.dram_tensor("xT", [C, T], f32r, kind="ExternalInput").ap()
    wq = nc.dram_tensor("wq", [HPC, C, C], f32r, kind="ExternalInput").ap()
    wk = nc.dram_tensor("wk", [HPC, C, C], f32r, kind="ExternalInput").ap()
    wv = nc.dram_tensor("wv", [C, HPC * C], f32r, kind="ExternalInput").ap()
    wfc = nc.dram_tensor("wfc", [HPC, C, OUT], f32r, kind="ExternalInput").ap()
    masks = nc.dram_tensor("masks", [SB, SB], bf16, kind="ExternalInput").ap()
    y = nc.dram_tensor("y", [OUT, T], f32, kind="ExternalOutput").ap()

    with tile.TileContext(nc) as tc:
        with (
            tc.tile_pool(name="const", bufs=1) as const,
            tc.tile_pool(name="big", bufs=1) as big,
            tc.tile_pool(name="super", bufs=2, space="PSUM") as psup,
            tc.tile_pool(name="psacc", bufs=1, space="PSUM") as psacc,
            tc.tile_pool(name="psy", bufs=2, space="PSUM") as psy,
            tc.tile_pool(name="exp", bufs=10) as expp,
            tc.tile_pool(name="small", bufs=3) as small,
            _repeat_loop(tc, mybir, repeat),
        ):
            # ---- constants / inputs to SBUF ----
            xT_sb = big.tile([C, T], f32r, tag="xT")
            nc.sync.dma_start(xT_sb[:], xT[:])
            wq_sb = const.tile([C, HPC, C], f32r, tag="wq")
            wk_sb = const.tile([C, HPC, C], f32r, tag="wk")
            wfc_sb = const.tile([C, HPC, OUT], f32r, tag="wfc")
            for h in range(HPC):
                nc.sync.dma_start(wq_sb[:, h, :], wq[h])
                nc.sync.dma_start(wk_sb[:, h, :], wk[h])
                nc.sync.dma_start(wfc_sb[:, h, :], wfc[h])
            wv_sb = const.tile([C, HPC * C], f32r, tag="wv")
            nc.sync.dma_start(wv_sb[:], wv[:])
            mask_sb = const.tile([SB, SB], bf16, tag="mask")
            nc.sync.dma_start(mask_sb[:], masks[:])
            ones_sb = const.tile([SB, SB], bf16, tag="ones")
            nc.vector.memset(ones_sb[:], 1.0)

            # ---- projections ----
            # v: natural layout (t-part, [heads, c]-free), bf16
            v_sb = big.tile([SB, T // SB, HPC, SB], bf16, tag="v")
            for sv in range(T // SB // GW):
                ps = psup.tile([C, GW * HPC * SB], f32, tag="super")
                for u in range(GW):
                    tb = GW * sv + u
                    nc.tensor.matmul(
                        ps[:, u * 512:(u + 1) * 512],
                        lhsT=xT_sb[:, tb * SB:(tb + 1) * SB],
                        rhs=wv_sb[:],
                        start=True, stop=True,
                    )
                nc.vector.tensor_copy(v_sb[:, GW * sv:GW * (sv + 1), :, :], ps[:])

            # qT / kT per head (c'-part, t-free), fp32r
            qT_sb = [big.tile([C, T], f32r, tag=f"qT{h}", name=f"qT{h}")
                     for h in range(HPC)]
            kT_sb = [big.tile([C, T], f32r, tag=f"kT{h}", name=f"kT{h}")
                     for h in range(HPC)]
            for h in range(HPC):
                for w_sb, dst, eng in ((wq_sb, qT_sb[h], nc.vector),
                                       (wk_sb, kT_sb[h], None)):
                    for sv in range(2):
                        ps = psup.tile([C, 1024], f32, tag="super")
                        for u in range(2):
                            tci = 2 * sv + u
                            nc.tensor.matmul(
                                ps[:, u * TCW:(u + 1) * TCW],
                                lhsT=w_sb[:, h, :],
                                rhs=xT_sb[:, tci * TCW:(tci + 1) * TCW],
                                start=True, stop=True,
                            )
                        dslice = dst[:, sv * 1024:(sv + 1) * 1024]
                        if eng is None:
                            # kT copies via ScalarE to offload the DVE
                            nc.scalar.copy(dslice, ps[:])
                        else:
                            eng.tensor_copy(dslice, ps[:])

            # ---- attention + fc ----
            # Per (t-chunk, head): full s-blocks (si < 4*tci) in pairs per
            # (128,1024) PSUM super-tile; the 4 diagonal s-blocks (si = 4tci+r)
            # are trimmed to their causally-valid t range [128r:512) (scores
            # matmul width floored at 256 to keep fp32r at full rate) and only
            # the leading 128 columns (the triangular block) get masked.
            for tci in range(NTC):
                yps = psy.tile([OUT, TCW], f32, tag="yps")
                for j in range(HPC):
                    n_si = 4 * tci + 4
                    qchunk = qT_sb[j][:, tci * TCW:(tci + 1) * TCW]
                    # (ex_tile, ex_slice_start, valid_width) per s-block
                    work = []
                    if with_scores:
                        for G in range(tci * 4 // GW):  # full s-block pairs
                            ps = psup.tile([C, GW * TCW], f32, tag="super")
                            for u in range(GW):
                                si = GW * G + u
                                nc.tensor.matmul(
                                    ps[:, u * TCW:(u + 1) * TCW],
                                    lhsT=kT_sb[j][:, si * SB:(si + 1) * SB],
                                    rhs=qchunk,
                                    start=True, stop=True,
                                )
                            ex = expp.tile([C, GW * TCW], bf16, tag="exp")
                            nc.scalar.activation(out=ex[:], in_=ps[:], func=EXPF,
                                                 scale=INV_SQRT_C)
                            for u in range(GW):
                                work.append((ex, u * TCW, TCW, GW * G + u))
                        for pair in range(2):  # diagonal s-blocks, trimmed
                            ps = psup.tile([C, GW * TCW], f32, tag="super")
                            for u in range(GW):
                                r = 2 * pair + u
                                si = 4 * tci + r
                                n_s = max(TCW - SB * r, 256)
                                nc.tensor.matmul(
                                    ps[:, u * TCW:u * TCW + n_s],
                                    lhsT=kT_sb[j][:, si * SB:(si + 1) * SB],
                                    rhs=qT_sb[j][:, (tci + 1) * TCW - n_s:
                                                (tci + 1) * TCW],
                                    start=True, stop=True,
                                )
                                n_e = TCW - SB * r
                                ex = expp.tile([C, TCW], bf16, tag="exd",
                                               name=f"exd{tci}_{j}_{r}")
                                nc.scalar.activation(
                                    out=ex[:, :n_e],
                                    in_=ps[:, u * TCW + n_s - n_e:u * TCW + n_s],
                                    func=EXPF, scale=INV_SQRT_C)
                                nc.vector.tensor_mul(ex[:, :SB], ex[:, :SB],
                                                     mask_sb[:])
                                work.append((ex, 0, n_e, si))
                    # out' accumulation over s blocks
                    outp = psacc.tile([C, TCW], f32, tag="outp")
                    if with_av:
                        for k, (ex, off, width, si) in enumerate(work):
                            nc.tensor.matmul(
                                outp[:, TCW - width:],
                                lhsT=v_sb[:, si, j, :],
                                rhs=ex[:, off:off + width],
                                start=(k == 0), stop=(k == n_si - 1),
                            )
                    # denominator (broadcast over partitions via ones)
                    denp = psacc.tile([C, TCW], f32, tag="denp")
                    if with_den:
                        for k, (ex, off, width, si) in enumerate(work):
                            nc.tensor.matmul(
                                denp[:, TCW - width:],
                                lhsT=ones_sb[:],
                                rhs=ex[:, off:off + width],
                                start=(k == 0), stop=(k == n_si - 1),
                            )
                    # normalize:  outn = out' * (1/den)
                    outn = small.tile([C, TCW], f32r, tag="outn")
                    if with_den:
                        recip = small.tile([C, TCW], f32, tag="recip")
                        nc.vector.reciprocal(recip[:], denp[:])
                        nc.vector.tensor_mul(outn[:], outp[:], recip[:])
                    else:
                        nc.vector.tensor_copy(outn[:], outp[:])
                    # fc partial, accumulated across this core's heads
                    nc.tensor.matmul(
                        yps[:],
                        lhsT=wfc_sb[:, j, :],
                        rhs=outn[:],
                        start=(j == 0), stop=(j == HPC - 1),
                    )
                ysb = small.tile([OUT, TCW], f32, tag="ysb")
                nc.vector.tensor_copy(ysb[:], yps[:])
                nc.sync.dma_start(y[:, tci * TCW:(tci + 1) * TCW], ysb[:])

    nc.compile()
    return nc


def _in_maps(x, Wq, Wk, Wv, Wfc):
    masks = _make_masks()
    maps = []
    for core in range(NCORES):
        b, g = divmod(core, NCORES // B)
        heads = [HPC * g + j for j in range(HPC)]
        maps.append({
            "xT": np.ascontiguousarray(x[b].T),
            "wq": np.ascontiguousarray(np.stack([Wq[:, h::H] for h in heads])),
            "wk": np.ascontiguousarray(np.stack([Wk[:, h::H] for h in heads])),
            "wv": np.ascontiguousarray(
                np.concatenate([Wv[:, h::H] for h in heads], axis=1)),
            "wfc": np.ascontiguousarray(
                np.stack([Wfc[h * C:(h + 1) * C, :] for h in heads])),
            "masks": masks,
        })
    return maps


def kernel(**inputs):
    from concourse.bass_utils import run_bass_kernel_spmd

    x = np.asarray(inputs["x"], dtype=np.float32)
    Wq = np.asarray(inputs["Wq"], dtype=np.float32)
    Wk = np.asarray(inputs["Wk"], dtype=np.float32)
    Wv = np.asarray(inputs["Wv"], dtype=np.float32)
    Wfc = np.asarray(inputs["Wfc"], dtype=np.float32)
    bfc = np.asarray(inputs["bfc"], dtype=np.float32)

    if "nc" not in _CACHE:
        _CACHE["nc"] = _build()
    nc = _CACHE["nc"]

    res = run_bass_kernel_spmd(nc, _in_maps(x, Wq, Wk, Wv, Wfc),
                               core_ids=list(range(NCORES)))
    out = np.empty((B, T, OUT), dtype=np.float32)
    for b in range(B):
        acc = bfc[:, None].copy()
        for g in range(NCORES // B):
            acc = acc + res.results[(NCORES // B) * b + g]["y"]
        out[b] = acc.T
    return out


# revision 24
# speedup vs baseline: 3072.1506x; 1.0159x over previous
"""Trainium2 Bass kernel for nn_DecoderMultiHeadedAttention_58102317580621.

Multi-head decoder (causal) attention, B=2, T=2048, C=128 (per-head dim),
H=16 heads, fused qkv projections + causal softmax attention + output fc.

Sharding: core = b*4 + g owns batch b and heads [4g, 4g+4).  Each core
computes, for its 4 heads: qT/kT/v projections, causal-masked softmax
attention (transposed layout, denominator via ones-matmul), and the partial
fc contraction summed over its heads on-chip (PSUM).  Host sums the 4
per-core partials per batch and adds the bias.

Layouts (per core, all on-chip tensors have 128 partitions):
  xT      (C, T)   = x[b].T                    fp32r
  qT_h    (C, T)   = (x @ Wq_h).T  = Wq_h.T @ x.T   (matmul lhsT=Wq_h, rhs=xT)
  kT_h    (C, T)   analogous
  v       (T, C)   natural, bf16, 4 heads packed:  v[:, si, j, :]
  scoresT (s, t)   = kT_h[:, s].T @ qT_h[:, t]  (contraction over per-head c')
  exp     bf16, causal mask applied multiplicatively post-exp (no max
           subtraction: |logit| <= ~6 so exp is safe in fp32/bf16)
  out'    (C, t)   = sum_s v[s,c] * exp[s,t]   (PSUM accum over s blocks)
  den     (*, t)   = sum_s exp[s,t] broadcast over partitions via ones-matmul
  y'      (OUT, t) = sum_heads Wfc_h.T @ (out' * 1/den)   (PSUM accum)
"""

import numpy as np
import ml_dtypes

B, T, C, H, OUT = 2, 2048, 128, 16, 128
NCORES = 8
HPC = 4          # heads per core
TCW = 512        # t-chunk width
NTC = T // TCW   # 4
SB = 128         # s-block (PE contraction tile)
GW = 2           # s-blocks per score super-tile (128 x GW*TCW psum)
INV_SQRT_C = float(1.0 / np.sqrt(C))

_CACHE = {}


def _make_masks():
    # tri[p, t'] = 1 if p <= t' else 0  (triangle for the diagonal 128-block)
    p = np.arange(SB)[:, None]
    t = np.arange(SB)[None, :]
    return np.ascontiguousarray((p <= t).astype(ml_dtypes.bfloat16))


def _repeat_loop(tc, mybir, repeat):
    # Timing builds wrap the whole body in a hardware loop so per-iteration
    # device time can be measured from wall-clock deltas.
    import contextlib
    if repeat <= 1:
        return contextlib.nullcontext()
    return tc.For_i(0, repeat, 1,
                    hint_engines=(mybir.EngineType.PE,
                                  mybir.EngineType.Activation,
                                  mybir.EngineType.DVE))


def _build(repeat=1, with_den=True, with_av=True, with_scores=True,
           k_bf16=False, den_pack=False):
    import concourse.tile as tile
    from concourse import bacc, mybir

    f32 = mybir.dt.float32
    f32r = mybir.dt.float32r
    bf16 = mybir.dt.bfloat16
    EXPF = mybir.ActivationFunctionType.Exp

    nc = bacc.Bacc("TRN2", target_bir_lowering=False, debug=False,
                   num_devices=NCORES)
    xT = nc.dram_tensor("xT", [C, T], f32r, kind="ExternalInput").ap()
    wq = nc.dram_tensor("wq", [HPC, C, C], f32r, kind="ExternalInput").ap()
    wk = nc.dram_tensor("wk", [HPC, C, C], f32r, kind="ExternalInput").ap()
    wv = nc.dram_tensor("wv", [C, HPC * C], f32r, kind="ExternalInput").ap()
    wfc = nc.dram_tensor("wfc", [HPC, C, OUT], f32r, kind="ExternalInput").ap()
    masks = nc.dram_tensor("masks", [SB, SB], bf16, kind="ExternalInput").ap()
    y = nc.dram_tensor("y", [OUT, T], f32, kind="ExternalOutput").ap()

    with tile.TileContext(nc) as tc:
        with (
            tc.tile_pool(name="const", bufs=1) as const,
            tc.tile_pool(name="big", bufs=1) as big,
            tc.tile_pool(name="super", bufs=2, space="PSUM") as psup,
            tc.tile_pool(name="psacc", bufs=1, space="PSUM") as psacc,
            tc.tile_pool(name="psy", bufs=2, space="PSUM") as psy,
            tc.tile_pool(name="exp", bufs=16) as expp,
            tc.tile_pool(name="small", bufs=3) as small,
            _repeat_loop(tc, mybir, repeat),
        ):
            # ---- constants / inputs to SBUF ----
            xT_sb = big.tile([C, T], f32r, tag="xT")
            nc.sync.dma_start(xT_sb[:], xT[:])
            wq_sb = const.tile([C, HPC, C], f32r, tag="wq")
            wk_sb = const.tile([C, HPC, C], f32r, tag="wk")
            wfc_sb = const.tile([C, HPC, OUT], f32r, tag="wfc")
            for h in range(HPC):
                nc.sync.dma_start(wq_sb[:, h, :], wq[h])
                nc.sync.dma_start(wk_sb[:, h, :], wk[h])
                nc.sync.dma_start(wfc_sb[:, h, :], wfc[h])
            wv_sb = const.tile([C, HPC * C], f32r, tag="wv")
            nc.sync.dma_start(wv_sb[:], wv[:])
            mask_sb = const.tile([SB, SB], bf16, tag="mask")
            nc.sync.dma_start(mask_sb[:], masks[:])
            ones_sb = const.tile([SB, SB], bf16, tag="ones")
            nc.vector.memset(ones_sb[:], 1.0)
            onesf_sb = const.tile([SB, SB], f32, tag="onesf")
            nc.vector.memset(onesf_sb[:], 1.0)
            onesr_sb = const.tile([SB, SB], f32r, tag="onesr")
            nc.vector.tensor_copy(onesr_sb[:], onesf_sb[:])

            # ---- projections ----
            # v: natural layout (t-part, [heads, c]-free), bf16
            v_sb = big.tile([SB, T // SB, HPC, SB], bf16, tag="v")
            for sv in range(T // SB // GW):
                ps = psup.tile([C, GW * HPC * SB], f32, tag="super")
                for u in range(GW):
                    tb = GW * sv + u
                    nc.tensor.matmul(
                        ps[:, u * 512:(u + 1) * 512],
                        lhsT=xT_sb[:, tb * SB:(tb + 1) * SB],
                        rhs=wv_sb[:],
                        start=True, stop=True,
                    )
                nc.vector.tensor_copy(v_sb[:, GW * sv:GW * (sv + 1), :, :], ps[:])

            # qT / kT per head (c'-part, t-free), fp32r
            qkdt = bf16 if k_bf16 else f32r
            qT_sb = [big.tile([C, T], qkdt, tag=f"qT{h}", name=f"qT{h}")
                     for h in range(HPC)]
            kT_sb = [big.tile([C, T], qkdt, tag=f"kT{h}", name=f"kT{h}")
                     for h in range(HPC)]
            for h in range(HPC):
                for w_sb, dst in ((wq_sb, qT_sb[h]), (wk_sb, kT_sb[h])):
                    for sv in range(2):
                        ps = psup.tile([C, 1024], f32, tag="super")
                        for u in range(2):
                            tci = 2 * sv + u
                            nc.tensor.matmul(
                                ps[:, u * TCW:(u + 1) * TCW],
                                lhsT=w_sb[:, h, :],
                                rhs=xT_sb[:, tci * TCW:(tci + 1) * TCW],
                                start=True, stop=True,
                            )
                        nc.vector.tensor_copy(
                            dst[:, sv * 1024:(sv + 1) * 1024], ps[:])

            # ---- attention + fc ----
            # Per (t-chunk, head): full s-blocks (si < 4*tci) in pairs per
            # (128,1024) PSUM super-tile; the 4 diagonal s-blocks (si = 4tci+r)
            # are trimmed to their causally-valid t range [128r:512) (scores
            # matmul width floored at 256 to keep fp32r at full rate) and only
            # the leading 128 columns (the triangular block) get masked.
            for tci in range(NTC):
                yps = psy.tile([OUT, TCW], f32, tag="yps")
                n_si = 4 * tci + 4
                if den_pack:
                    # den4: partition strip [32j, 32j+32) holds head j's
                    # denominator, written by col-group-tiled M=32
                    # ones-matmuls (head pairs run concurrently in distinct
                    # col groups).
                    den4 = psacc.tile([C, TCW], f32, tag="den4",
                                      name=f"den4_{tci}")
                for pair in (0, 1):
                    jj = (2 * pair, 2 * pair + 1)
                    work = {j: [] for j in jj}
                    # scores + exp (+ triangular mask on diagonal blocks)
                    for j in jj:
                        if not with_scores:
                            continue
                        qchunk = qT_sb[j][:, tci * TCW:(tci + 1) * TCW]
                        for G in range(tci * 4 // GW):  # full s-block pairs
                            ps = psup.tile([C, GW * TCW], f32, tag="super")
                            for u in range(GW):
                                si = GW * G + u
                                nc.tensor.matmul(
                                    ps[:, u * TCW:(u + 1) * TCW],
                                    lhsT=kT_sb[j][:, si * SB:(si + 1) * SB],
                                    rhs=qchunk,
                                    start=True, stop=True,
                                )
                            ex = expp.tile([C, GW * TCW], bf16, tag="exp")
                            nc.scalar.activation(out=ex[:], in_=ps[:],
                                                 func=EXPF, scale=INV_SQRT_C)
                            for u in range(GW):
                                work[j].append((ex, u * TCW, TCW, GW * G + u))
                        for dp in range(2):  # diagonal s-blocks, trimmed
                            ps = psup.tile([C, GW * TCW], f32, tag="super")
                            for u in range(GW):
                                r = 2 * dp + u
                                si = 4 * tci + r
                                n_s = max(TCW - SB * r, 256)
                                nc.tensor.matmul(
                                    ps[:, u * TCW:u * TCW + n_s],
                                    lhsT=kT_sb[j][:, si * SB:(si + 1) * SB],
                                    rhs=qT_sb[j][:, (tci + 1) * TCW - n_s:
                                                (tci + 1) * TCW],
                                    start=True, stop=True,
                                )
                                n_e = TCW - SB * r
                                ex = expp.tile([C, TCW], bf16, tag="exd", bufs=12,
                                               name=f"exd{tci}_{j}_{r}")
                                nc.scalar.activation(
                                    out=ex[:, :n_e],
                                    in_=ps[:, u * TCW + n_s - n_e:
                                            u * TCW + n_s],
                                    func=EXPF, scale=INV_SQRT_C)
                                nc.vector.tensor_mul(ex[:, :SB], ex[:, :SB],
                                                     mask_sb[:])
                                work[j].append((ex, 0, n_e, si))
                    # denominator strips, 2 heads col-packed per s-block
                    if with_den and with_scores and den_pack:
                        for k in range(n_si):
                            for j in jj:
                                ex, off, width, si = work[j][k]
                                nc.tensor.matmul(
                                    den4[32 * j:32 * (j + 1), TCW - width:],
                                    lhsT=ones_sb[:, 0:32],
                                    rhs=ex[:, off:off + width],
                                    start=(k == 0), stop=(k == n_si - 1),
                                    tile_position=(0, 32 * j),
                                )
                    # AV + normalize + fc per head
                    for j in jj:
                        outp = psacc.tile([C, TCW], f32, tag="outp",
                                          name=f"outp{tci}_{j}")
                        if with_av and with_scores:
                            for k, (ex, off, width, si) in enumerate(work[j]):
                                nc.tensor.matmul(
                                    outp[:, TCW - width:],
                                    lhsT=v_sb[:, si, j, :],
                                    rhs=ex[:, off:off + width],
                                    start=(k == 0), stop=(k == n_si - 1),
                                )
                        else:
                            nc.vector.memset(outp[:], 0.0)
                        outn = small.tile([C, TCW], f32r, tag="outn")
                        if with_den and with_scores and den_pack:
                            # reciprocal of this head's strip, broadcast to all
                            # 128 partitions via a K=32 ones matmul
                            rsm = small.tile([C, TCW], f32r, tag="rsm")
                            with nc.allow_low_precision(
                                    reason="f32r reciprocal, same bits as f32"):
                                nc.vector.reciprocal(
                                    rsm[32 * j:32 * (j + 1), :],
                                    den4[32 * j:32 * (j + 1), :])
                            bc = psup.tile([C, TCW], f32, tag="super",
                                           name=f"bc{tci}_{j}")
                            nc.tensor.matmul(
                                bc[:],
                                lhsT=onesr_sb[32 * j:32 * (j + 1), :],
                                rhs=rsm[32 * j:32 * (j + 1), :],
                                start=True, stop=True,
                                tile_position=(32 * j, 0),
                            )
                            rbc = small.tile([C, TCW], f32, tag="rbc")
                            nc.vector.tensor_copy(rbc[:], bc[:])
                            nc.vector.tensor_mul(outn[:], outp[:], rbc[:])
                        elif with_den and with_scores:
                            # M=128 ones matmul denominator (broadcast built
                            # into the matmul output), per head
                            denp = psacc.tile([C, TCW], f32, tag="den4",
                                              name=f"denp{tci}_{j}")
                            for k, (ex, off, width, si) in enumerate(work[j]):
                                nc.tensor.matmul(
                                    denp[:, TCW - width:],
                                    lhsT=ones_sb[:],
                                    rhs=ex[:, off:off + width],
                                    start=(k == 0), stop=(k == n_si - 1),
                                )
                            recip = small.tile([C, TCW], f32, tag="recip")
                            nc.vector.reciprocal(recip[:], denp[:])
                            nc.vector.tensor_mul(outn[:], outp[:], recip[:])
                        else:
                            nc.vector.tensor_copy(outn[:], outp[:])
                        # fc partial, accumulated across this core's heads
                        nc.tensor.matmul(
                            yps[:],
                            lhsT=wfc_sb[:, j, :],
                            rhs=outn[:],
                            start=(j == 0), stop=(j == HPC - 1),
                        )
                ysb = small.tile([OUT, TCW], f32, tag="ysb")
                nc.vector.tensor_copy(ysb[:], yps[:])
                nc.sync.dma_start(y[:, tci * TCW:(tci + 1) * TCW], ysb[:])

    nc.compile()
    return nc


def _in_maps(x, Wq, Wk, Wv, Wfc):
    masks = _make_masks()
    maps = []
    for core in range(NCORES):
        b, g = divmod(core, NCORES // B)
        heads = [HPC * g + j for j in range(HPC)]
        maps.append({
            "xT": np.ascontiguousarray(x[b].T),
            "wq": np.ascontiguousarray(np.stack([Wq[:, h::H] for h in heads])),
            "wk": np.ascontiguousarray(np.stack([Wk[:, h::H] for h in heads])),
            "wv": np.ascontiguousarray(
                np.concatenate([Wv[:, h::H] for h in heads], axis=1)),
            "wfc": np.ascontiguousarray(
                np.stack([Wfc[h * C:(h + 1) * C, :] for h in heads])),
            "masks": masks,
        })
    return maps


def kernel(**inputs):
    from concourse.bass_utils import run_bass_kernel_spmd

    x = np.asarray(inputs["x"], dtype=np.float32)
    Wq = np.asarray(inputs["Wq"], dtype=np.float32)
    Wk = np.asarray(inputs["Wk"], dtype=np.float32)
    Wv = np.asarray(inputs["Wv"], dtype=np.float32)
    Wfc = np.asarray(inputs["Wfc"], dtype=np.float32)
    bfc = np.asarray(inputs["bfc"], dtype=np.float32)

    if "nc" not in _CACHE:
        _CACHE["nc"] = _build()
    nc = _CACHE["nc"]

    res = run_bass_kernel_spmd(nc, _in_maps(x, Wq, Wk, Wv, Wfc),
                               core_ids=list(range(NCORES)))
    out = np.empty((B, T, OUT), dtype=np.float32)
    for b in range(B):
        acc = bfc[:, None].copy()
        for g in range(NCORES // B):
            acc = acc + res.results[(NCORES // B) * b + g]["y"]
        out[b] = acc.T
    return out
